# revision 1
# baseline (speedup 1.0000x reference)
"""CapsuleLayer (dynamic routing) Trainium2 kernel, SPMD over 8 NeuronCores.

Sharding: input-capsule axis (IN_CAPS=512 -> 64 per core). W and u_hat are
i-sharded; the bij,bijd->bjd contraction is completed with an AllReduce of
bf16 s-partials (2x64x16x32, 128 KB) once per routing iteration.

Per-core layout (i_local = i2*32 + i1, i2 in {0,1}):
  u_hat SBUF [p=(i2*64+b), (d, i1, j)] bf16 -- 128 partitions x 16384
  b/c logits [p, (i1, j)], s/outputs [b, (d, j)].

The (d, i1, j) free order keeps every big DVE pass in the bf16 2x perf mode:
both broadcast multiplies broadcast over a non-innermost dim (innermost stays
step-1), and both reductions are in-place contiguous tree-adds.

Changes over the original baseline (252-347us -> ~205-250us; most of the
remaining time is the uncontrollable collective-stack init barrier, 30-50us,
plus 3 x ~12.5us AllReduce latency and a ~33us/iteration DVE floor):
- W streamed in 1 MB chunks (2 groups per DMA, host layout pair-blocked).
- AllReduce payloads are bf16 (128 KB instead of 256 KB f32): ~2.5us/AR.
- AR results are DMA'd into BOTH partition halves (4 small returns on two
  HWDGE queues) so the squash chain runs on 128 partitions and the
  bu-multiply needs no mid-chain partition-copy DMA.
- sqrt(ss+eps) is computed as exp(0.5*ln(ss+eps)) and the activation tables
  are pinned to the one set containing Ln+Exp+Copy, so the per-iteration
  ACT_TABLE_LOAD thrash (Exp<->Sqrt, ~1.3us each on the critical path)
  disappears.
- squash-scale is produced in bf16 so the out2/bu multiplies stay in the
  DVE 2x mode.

Measured notes (for future iterations): DVE tensor_tensor bf16 runs at 2x
(0.53 ns/elem) ONLY for full-width operands; slicing an outer dim drops
in-place tree-adds to ~1x and even out-of-place to ~0.9x, so the fold trees
must stay full-width. GpSimd tensor ops measure 1.9 ns/elem on flat APs but
~4.7 ns/elem on sliced/broadcast APs, which makes d-splitting the big
multiplies between vector and gpsimd a net loss. GPSIMD cannot read PSUM.
remote_dma/remote_dma_broadcast compile but hang this runtime (axon
fake_nrt), so the collective stack is unavoidable.

Phase 1 (per i): u_hat_i[b, dj] = xT_i.T @ W_i on the PE (K=128, M=64,
N=512), all in bf16. s partials are AllReduced per partition-half.
"""

import numpy as np

N_CORES = 8
B = 64
IN_CAPS = 512
IN_DIM = 128
N_CAPS = 16
OUT_DIM = 32
I_LOC = IN_CAPS // N_CORES          # 64 input capsules per core
I1 = 32                             # i_local = i2*32 + i1
JD = N_CAPS * OUT_DIM               # 512
EPS = 1e-7
GRP = 4                             # i's per PSUM group
NGRP = I_LOC // GRP                 # 16
NPAIR = NGRP // 2                   # 8 (two groups per W DMA)

# Toggled by test.py for profiling runs.
TRACE = False
TRACE_DIR = None

_cache = {}


def _emit(tc, xT, wT, out, num_routing):
    from contextlib import ExitStack

    from concourse import mybir

    nc = tc.nc
    f32 = mybir.dt.float32
    bf16 = mybir.dt.bfloat16
    ctx = ExitStack()
    singles = ctx.enter_context(tc.tile_pool(name="singles", bufs=1))
    wpool = ctx.enter_context(tc.tile_pool(name="wpool", bufs=4))
    pspool = ctx.enter_context(tc.tile_pool(name="pspool", bufs=2, space="PSUM"))
    small = ctx.enter_context(tc.tile_pool(name="small", bufs=2))
    dram = ctx.enter_context(tc.tile_pool(name="dram", bufs=2, space="DRAM"))

    # One tiny warmup collective: the collective stack finishes its background
    # init ~60-150us into the kernel and charges a first-collective premium
    # (~12us); a 128-byte AllReduce absorbs the premium off the critical path.
    warm_in = dram.tile([1, 32], f32)
    warm_out = dram.tile([1, 32], f32)
    nc.gpsimd.collective_compute(
        "AllReduce",
        mybir.AluOpType.add,
        replica_groups=[list(range(N_CORES))],
        ins=[warm_in.opt()],
        outs=[warm_out.opt()],
    )

    # ---- phase 1: u_hat = einsum over k, per local capsule i ----
    xsb = singles.tile([IN_DIM, I_LOC, B], bf16)         # [k, i, b]
    u_hat = singles.tile([128, OUT_DIM, I1, N_CAPS], bf16)  # [(i2,b), d, i1, j]

    # cast-engine alternation: GPSIMD cannot read PSUM, so alternate
    # vector/scalar (both are hidden under the W DMA stream anyway)
    SC_CAST = {1, 3, 5, 7, 9, 11, 13, 15}

    XCH = I_LOC // 4
    for p in range(NPAIR):
        # interleave x chunks with the first W pairs so the first matmul's
        # operands land as early as possible. All loads stay on the sync
        # queue: issuing them from the scalar queue stalls the scalar casts
        # behind wpool waits (engine queues are in-order).
        if p < 4:
            nc.sync.dma_start(
                xsb[:, p * XCH:(p + 1) * XCH, :],
                xT[:, p * XCH:(p + 1) * XCH, :],
            )
        wtile = wpool.tile([IN_DIM, 2, GRP, OUT_DIM, N_CAPS], bf16)
        nc.sync.dma_start(wtile[:], wT[p])
        for g2 in range(2):
            g = 2 * p + g2
            i2 = (g * GRP) // I1
            i1g = (g * GRP) % I1
            ps = pspool.tile([128, GRP, OUT_DIM, N_CAPS], f32)
            for t in range(GRP):
                i = g * GRP + t
                nc.tensor.matmul(
                    ps[i2 * B:(i2 + 1) * B, t], xsb[:, i, :], wtile[:, g2, t],
                    start=True, stop=True,
                )
            # copy+cast PSUM f32 -> SBUF bf16; dst viewed (i1, d, j)
            dst = u_hat[i2 * B:(i2 + 1) * B, :, i1g:i1g + GRP, :].transpose(
                [0, 2, 1, 3]
            )
            src = ps[i2 * B:(i2 + 1) * B]
            if g in SC_CAST:
                nc.scalar.copy(out=dst, in_=src)
            else:
                nc.vector.tensor_copy(out=dst, in_=src)

    # ---- phase 2: routing ----
    tmp = singles.tile([128, OUT_DIM, I1, N_CAPS], bf16)
    b_log = singles.tile([128, I1, N_CAPS], f32)
    eps_t = singles.tile([128, 1], f32)
    nc.vector.memset(b_log[:], 0.0)
    nc.vector.memset(eps_t[:], EPS)

    R = num_routing
    for r in range(R):
        if r == 0:
            # b == 0 -> c uniform: s = (1/16) * sum_i u_hat (scale after AR)
            nc.vector.tensor_add(
                tmp[:, :, :I1 // 2], u_hat[:, :, :I1 // 2],
                u_hat[:, :, I1 // 2:],
            )
        else:
            # |b| stays < ~20 for this distribution: exp is fp32-safe
            # without the max-subtraction
            # cexp and the softmax reciprocal live in bf16 so the c_t
            # normalize runs in the DVE 2x mode (reduce accumulates in f32
            # internally; the 0.4% bf16 rounding on c washes out over the
            # 512-term i-sum)
            cexp = small.tile([128, I1, N_CAPS], bf16)
            nc.scalar.activation(
                out=cexp[:], in_=b_log[:],
                func=mybir.ActivationFunctionType.Exp,
            )
            csum = small.tile([128, I1], f32)
            with nc.allow_low_precision(
                reason="16-term bf16 sum into f32; 0.4% noise on c"
            ):
                nc.vector.reduce_sum(
                    out=csum[:], in_=cexp[:], axis=mybir.AxisListType.X
                )
            csum_r = small.tile([128, I1], bf16)
            with nc.allow_low_precision(
                reason="bf16 softmax reciprocal; 0.4% noise on c"
            ):
                nc.vector.reciprocal(out=csum_r[:], in_=csum[:])
            c_t = small.tile([128, I1, N_CAPS], bf16)
            nc.vector.tensor_mul(
                c_t[:], cexp[:],
                csum_r.unsqueeze(2).broadcast_to([128, I1, N_CAPS]),
            )
            # s-mul: broadcast c over outermost d keeps bf16 2x mode
            nc.vector.tensor_mul(
                tmp[:], u_hat[:],
                c_t.unsqueeze(1).broadcast_to([128, OUT_DIM, I1, N_CAPS]),
            )
            nc.vector.tensor_add(
                tmp[:, :, :I1 // 2], tmp[:, :, :I1 // 2], tmp[:, :, I1 // 2:]
            )
        # contiguous full-width in-place tree over i1 (middle dim): keeps
        # the DVE in bf16 2x mode (sliced variants drop to ~1x)
        w = I1 // 2
        while w > 2:
            nc.vector.tensor_add(
                tmp[:, :, :w // 2], tmp[:, :, :w // 2], tmp[:, :, w // 2:w]
            )
            w //= 2
        s_half = small.tile([128, OUT_DIM, N_CAPS], bf16)
        nc.vector.tensor_add(s_half[:], tmp[:, :, 0, :], tmp[:, :, 1, :])

        # AllReduce both partition halves in bf16; sum them after (engines
        # cannot shift partitions, so pre-folding would need an extra DMA)
        cc_in = dram.tile([2, B, OUT_DIM, N_CAPS], bf16)
        cc_out = dram.tile([2, B, OUT_DIM, N_CAPS], bf16)
        nc.sync.dma_start(cc_in[:], s_half[:])
        nc.gpsimd.collective_compute(
            "AllReduce",
            mybir.AluOpType.add,
            replica_groups=[list(range(N_CORES))],
            ins=[cc_in.opt()],
            outs=[cc_out.opt()],
        )
        last = r == R - 1
        # return each half into BOTH partition halves so everything
        # downstream runs on 128 partitions with no partition-copy
        NP = B if last else 128
        s_lo = small.tile([NP, OUT_DIM, N_CAPS], bf16)
        s_hi = small.tile([NP, OUT_DIM, N_CAPS], bf16)
        nc.sync.dma_start(s_lo[0:B], cc_out[0])
        nc.scalar.dma_start(s_hi[0:B], cc_out[1])
        if not last:
            nc.scalar.dma_start(s_lo[B:2 * B], cc_out[0])
            nc.sync.dma_start(s_hi[B:2 * B], cc_out[1])
        s_sb = small.tile([NP, OUT_DIM, N_CAPS], bf16)
        nc.vector.tensor_add(s_sb[:], s_lo[:], s_hi[:])
        # iteration 0's uniform c = 1/16 is folded into the squash math:
        # with s' = 16*s, ss = sum_d (s'/16)^2 and out = f(ss) * (s'/16)
        inv = 1.0 / N_CAPS if r == 0 else 1.0

        # squash: scale = ss/(1+ss)/sqrt(ss+eps), ss = sum_d s^2
        # sqrt(x) = exp(0.5*ln(x)): keeps Ln/Exp/Copy in ONE activation
        # table set (no per-iteration table reloads)
        sq = small.tile([NP, OUT_DIM, N_CAPS], bf16)
        nc.vector.scalar_tensor_tensor(
            sq[:], s_sb[:], inv * inv, s_sb[:],
            mybir.AluOpType.mult, mybir.AluOpType.mult,
        )
        ss = small.tile([NP, N_CAPS], f32)
        with nc.allow_low_precision(
            reason="32-term bf16 square-sum into f32; 0.2% on squash scale"
        ):
            nc.vector.reduce_sum(
                out=ss[:], in_=sq.transpose([0, 2, 1]),
                axis=mybir.AxisListType.X,
            )
        t1 = small.tile([NP, N_CAPS], f32)
        nc.scalar.activation(
            out=t1[:], in_=ss[:], func=mybir.ActivationFunctionType.Ln,
            bias=eps_t[0:NP], scale=1.0,
        )
        tq = small.tile([NP, N_CAPS], f32)
        nc.scalar.activation(
            out=tq[:], in_=t1[:], func=mybir.ActivationFunctionType.Exp,
            bias=0.0, scale=0.5,
        )   # sqrt(ss+eps)
        t2 = small.tile([NP, N_CAPS], f32)
        nc.vector.scalar_tensor_tensor(
            t2[:], ss[:], 1.0, tq[:],
            mybir.AluOpType.add, mybir.AluOpType.mult,
        )   # (1+ss)*sqrt(ss+eps)
        nc.vector.reciprocal(out=t2[:], in_=t2[:])
        t1b = small.tile([NP, N_CAPS], bf16)
        nc.vector.scalar_tensor_tensor(
            t1b[:], ss[:], inv, t2[:],
            mybir.AluOpType.mult, mybir.AluOpType.mult,
        )   # squash scale (*inv), bf16 so the out2 mul stays in 2x mode
        if last:
            # write the (j, d)-ordered output directly via a transposed AP
            out_t = small.tile([B, N_CAPS, OUT_DIM], f32)
            nc.vector.tensor_mul(
                out_t.transpose([0, 2, 1]), s_sb[:],
                t1b.unsqueeze(1).broadcast_to([B, OUT_DIM, N_CAPS]),
            )
            nc.sync.dma_start(out[:], out_t[:])
        else:
            out2 = small.tile([128, OUT_DIM, N_CAPS], bf16)
            nc.vector.tensor_mul(
                out2[:], s_sb[:],
                t1b.unsqueeze(1).broadcast_to([128, OUT_DIM, N_CAPS]),
            )
            # bu-mul: broadcast outputs over middle i1 keeps bf16 2x mode
            nc.vector.tensor_mul(
                tmp[:], u_hat[:],
                out2.unsqueeze(2).broadcast_to([128, OUT_DIM, I1, N_CAPS]),
            )
            w = OUT_DIM
            while w > 2:
                nc.vector.tensor_add(
                    tmp[:, :w // 2], tmp[:, :w // 2], tmp[:, w // 2:w]
                )
                w //= 2
            bred = small.tile([128, I1, N_CAPS], bf16)
            nc.vector.tensor_add(bred[:], tmp[:, 0], tmp[:, 1])
            nc.vector.tensor_add(b_log[:], b_log[:], bred[:])

    ctx.close()


class _single_act_table:
    """Make every activation resolve to the one table set that covers
    Exp+Ln+Copy (natural_log_exp_and_others), so the kernel loads activation
    tables exactly once instead of thrashing Exp<->Ln sets (~1.3us per
    reload, on the critical path). Positional set ids are preserved, so the
    walrus side (which indexes the same act_info.json) stays consistent.
    Scoped: restores the original resolver on exit."""

    def __enter__(self):
        import concourse.bacc as bacc

        self._bacc = bacc
        self._orig = orig = bacc.get_activation_tables

        def patched(arch):
            tables = dict(orig(arch))
            keep = "natural_log_exp_and_others"
            if keep in tables:
                for k in tables:
                    if k != keep:
                        tables[k] = set()
            return tables

        bacc.get_activation_tables = patched
        return self

    def __exit__(self, *exc):
        self._bacc.get_activation_tables = self._orig
        return False


def _build(num_routing):
    import concourse.bacc as bacc
    import concourse.tile as tile
    from concourse import mybir

    nc = bacc.Bacc(
        "TRN2", target_bir_lowering=False, debug=False, num_devices=N_CORES,
        dynamic_dma_scratch_size=512,
    )
    f32 = mybir.dt.float32
    bf16 = mybir.dt.bfloat16
    xT = nc.dram_tensor("xT", [IN_DIM, I_LOC, B], bf16, kind="ExternalInput")
    wT = nc.dram_tensor(
        "wT", [NPAIR, IN_DIM, 2, GRP, OUT_DIM, N_CAPS], bf16,
        kind="ExternalInput",
    )
    out = nc.dram_tensor(
        "out", [B, N_CAPS, OUT_DIM], f32, kind="ExternalOutput"
    )
    with tile.TileContext(nc) as tc:
        _emit(tc, xT, wT, out, num_routing)
    with _single_act_table():
        nc.compile()
    return nc


def kernel(inputs, W, num_routing):
    import ml_dtypes

    from concourse.bass_utils import run_bass_kernel_spmd

    R = int(num_routing)
    assert R >= 1
    if R not in _cache:
        _cache[R] = _build(R)
    nc = _cache[R]

    bf = ml_dtypes.bfloat16
    inputs = np.ascontiguousarray(np.asarray(inputs, dtype=np.float32))
    W = np.asarray(W, dtype=np.float32)

    in_maps = []
    for c in range(N_CORES):
        lo, hi = c * I_LOC, (c + 1) * I_LOC
        xT_c = np.ascontiguousarray(
            inputs[:, lo:hi, :].transpose(2, 1, 0).astype(bf)
        )
        # [i,j,k,d] -> pair-blocked [p, k, g2, t, d, j]: each 2-group DMA is
        # one contiguous 1MB block with 8KB contiguous per partition line
        wT_c = np.ascontiguousarray(
            W[lo:hi]
            .reshape(NPAIR, 2, GRP, N_CAPS, IN_DIM, OUT_DIM)
            .transpose(0, 4, 1, 2, 5, 3)
            .astype(bf)
        )
        in_maps.append({"xT": xT_c, "wT": wT_c})

    kwargs = {}
    if TRACE:
        kwargs["trace"] = True
        if TRACE_DIR:
            kwargs["tmpdir"] = TRACE_DIR
    res = None
    for attempt in range(3):
        try:
            res = run_bass_kernel_spmd(
                nc, in_maps, core_ids=list(range(N_CORES)), **kwargs
            )
            break
        except Exception:
            if attempt == 2:
                raise
            import time
            time.sleep(5)
    if TRACE:
        kernel.last_exec_time_ns = res.exec_time_ns
        kernel.last_results = res
    return np.asarray(res.results[0]["out"], dtype=np.float32)



# revision 3
# speedup vs baseline: 1.0494x; 1.0494x over previous
"""CapsuleLayer (dynamic routing) Trainium2 kernel, SPMD over 8 NeuronCores.

Sharding: input-capsule axis (IN_CAPS=512 -> 64 per core). W and u_hat are
i-sharded; the bij,bijd->bjd contraction is completed with AllReduces of
bf16 s-partials once per routing iteration.

Per-core layout (i_local = i2*32 + i1, i2 in {0,1}):
  u_hat SBUF [p=(i2*64+b), (d, i1, j)] bf16 -- 128 partitions x 16384
  b/c logits [p, (i1, j)], s partials [p, (d, j)].

v2 structure (over the 245us baseline):
- No warmup AllReduce. Instead, the r=0 s-partial (c uniform -> s0 =
  (1/16) sum_i u_hat, the 1/16 folded into the squash scale) is built
  incrementally DURING phase 1: each 4-i group is folded 4->1 as its
  PSUM cast lands, staged into gfs[128, 8, (d,j)], and tree-folded 8->1
  right after the last cast. Its AllReduce triggers at ~46us, so the AR
  starts the moment the NRT collective-init barrier (48-145us, run
  variance) ends, instead of queueing behind a warmup AR + DVE fold
  (~36us of serialized warmup+AR0 on the baseline critical path).
- Every s AllReduce is split into two d-halves (64 KB bf16 each) on the
  single CC stream. Downstream work is d-decomposable: the agreement
  update b_log += sum_d out*u_hat = scale[b,j] * sum_d s*u_hat (squash
  scale applied AFTER the d-fold, algebraically identical), so the
  bu-mul+fold for half A runs while half B's AR is still in flight.
  Per-iteration AR exposure drops from ~12.5us + full serial DVE to
  mostly-hidden.
- cc buffers are dedicated (bufs = exact tile count) so no DRAM-pool
  aliasing creates false WAR semaphore deps on the AR triggers.
- sqrt(ss+eps) = exp(0.5*ln(ss+eps)) with activation tables pinned to
  the one set containing Ln+Exp+Copy (no per-iteration table reloads).

Measured notes (from traces): DVE tensor_tensor bf16 runs 2x only with
step-1 innermost and full-width ops; broadcast over a non-innermost dim
keeps 2x. AR latency is fixed ~11-13us nearly independent of payload
(128B vs 128KB), so splitting costs stream time but buys overlap. The
NRT barrier (collective stack init) ends 48-145us into the kernel and
gates the first AR; nothing in-kernel controls it. GPSIMD ~1.9ns/elem
flat, ~4.7 sliced/broadcast. remote_dma hangs this runtime (axon
fake_nrt), so the collective stack is unavoidable.

Phase 1 (per i): u_hat_i[b, dj] = xT_i.T @ W_i on the PE (K=128, M=64,
N=512), all in bf16, W streamed in 1 MB pair-blocked chunks.
"""

import numpy as np

N_CORES = 8
B = 64
IN_CAPS = 512
IN_DIM = 128
N_CAPS = 16
OUT_DIM = 32
I_LOC = IN_CAPS // N_CORES          # 64 input capsules per core
I1 = 32                             # i_local = i2*32 + i1
DH = OUT_DIM // 2                   # 16, d-half for split ARs
EPS = 1e-7
GRP = 4                             # i's per PSUM group
NGRP = I_LOC // GRP                 # 16
NPAIR = NGRP // 2                   # 8 (two groups per W DMA)

# Toggled by test.py for profiling runs.
TRACE = False
TRACE_DIR = None

_cache = {}


def _emit(tc, xT, wT, out, num_routing):
    from contextlib import ExitStack

    from concourse import mybir

    nc = tc.nc
    f32 = mybir.dt.float32
    bf16 = mybir.dt.bfloat16
    R = num_routing
    ctx = ExitStack()
    singles = ctx.enter_context(tc.tile_pool(name="singles", bufs=1))
    wpool = ctx.enter_context(tc.tile_pool(name="wpool", bufs=4))
    pspool = ctx.enter_context(tc.tile_pool(name="pspool", bufs=2, space="PSUM"))
    small = ctx.enter_context(tc.tile_pool(name="small", bufs=2))
    gpool = ctx.enter_context(tc.tile_pool(name="gpool", bufs=2))
    # one slot per cc buffer: zero reuse -> no false WAR deps on triggers
    ccpool = ctx.enter_context(tc.tile_pool(name="ccpool", bufs=4 * R, space="DRAM"))

    cc_in = [[None, None] for _ in range(R)]
    cc_out = [[None, None] for _ in range(R)]
    for r in range(R):
        for h in (0, 1):
            cc_in[r][h] = ccpool.tile(
                [2, B, DH, N_CAPS], bf16, name=f"ccin{r}{h}"
            )
            cc_out[r][h] = ccpool.tile(
                [2, B, DH, N_CAPS], bf16, name=f"ccout{r}{h}"
            )

    def allreduce(r, h):
        nc.gpsimd.collective_compute(
            "AllReduce",
            mybir.AluOpType.add,
            replica_groups=[list(range(N_CORES))],
            ins=[cc_in[r][h].opt()],
            outs=[cc_out[r][h].opt()],
        )

    # ---- phase 1: u_hat = einsum over k, per local capsule i; the r=0
    # s-partial sum_i u_hat accumulates alongside ----
    xsb = singles.tile([IN_DIM, I_LOC, B], bf16)         # [k, i, b]
    u_hat = singles.tile([128, OUT_DIM, I1, N_CAPS], bf16)  # [(i2,b), d, i1, j]
    gfs = singles.tile([128, NPAIR, OUT_DIM, N_CAPS], bf16)  # group partials
    eps_t = singles.tile([128, 1], f32)
    nc.vector.memset(eps_t[:], EPS)

    XCH = I_LOC // 4
    for p in range(NPAIR):
        # interleave x chunks with the first W pairs so the first matmul's
        # operands land as early as possible. All loads stay on the sync
        # queue (engine queues are in-order; scalar must not stall).
        if p < 4:
            nc.sync.dma_start(
                xsb[:, p * XCH:(p + 1) * XCH, :],
                xT[:, p * XCH:(p + 1) * XCH, :],
            )
        wtile = wpool.tile([IN_DIM, 2, GRP, OUT_DIM, N_CAPS], bf16)
        nc.sync.dma_start(wtile[:], wT[p])
        for g2 in range(2):
            g = 2 * p + g2
            i2 = (g * GRP) // I1
            i1g = (g * GRP) % I1
            gh = (g % NPAIR)  # staging slot within this i2-half
            ps = pspool.tile([128, GRP, OUT_DIM, N_CAPS], f32)
            for t in range(GRP):
                i = g * GRP + t
                nc.tensor.matmul(
                    ps[i2 * B:(i2 + 1) * B, t], xsb[:, i, :], wtile[:, g2, t],
                    start=True, stop=True,
                )
            # copy+cast PSUM f32 -> SBUF bf16 on the ACT engine (frees the
            # DVE for the incremental r0 fold); dst viewed (i1, d, j)
            dst = u_hat[i2 * B:(i2 + 1) * B, :, i1g:i1g + GRP, :].transpose(
                [0, 2, 1, 3]
            )
            nc.scalar.copy(out=dst, in_=ps[i2 * B:(i2 + 1) * B])
            # incremental r0 fold: this group's 4 i's -> gfs slot
            ug = u_hat[i2 * B:(i2 + 1) * B, :, i1g:i1g + GRP, :]
            gtmp = gpool.tile([B, OUT_DIM, 2, N_CAPS], bf16, name="gtmp")
            nc.vector.tensor_add(gtmp[:], ug[:, :, 0:2, :], ug[:, :, 2:4, :])
            nc.vector.tensor_add(
                gfs[i2 * B:(i2 + 1) * B, gh], gtmp[:, :, 0, :], gtmp[:, :, 1, :]
            )
    # tree-fold the 8 group partials per half -> acc0 = sum_i u_hat
    nc.vector.tensor_add(gfs[:, :4], gfs[:, :4], gfs[:, 4:8])
    nc.vector.tensor_add(gfs[:, :2], gfs[:, :2], gfs[:, 2:4])
    acc0 = singles.tile([128, OUT_DIM, N_CAPS], bf16)
    nc.vector.tensor_add(acc0[:], gfs[:, 0], gfs[:, 1])
    for h in (0, 1):
        nc.sync.dma_start(cc_in[0][h][:], acc0[:, h * DH:(h + 1) * DH, :])
        allreduce(0, h)

    # ---- phase 2: routing, d-split pipelined around the ARs ----
    tmp = singles.tile([128, DH, I1, N_CAPS], bf16)
    b_log = singles.tile([128, I1, N_CAPS], f32)

    for r in range(R):
        last = r == R - 1
        NP = B if last else 128
        # iteration 0's uniform c = 1/16 is folded into the squash math:
        # with s' = 16*s, ss = sum_d (s'/16)^2 and out = f(ss) * (s'/16)
        inv = 1.0 / N_CAPS if r == 0 else 1.0

        s_h = [None, None]
        ss_h = [None, None]
        part = [None, None]
        for h in (0, 1):
            # AR(r) half-h result -> SBUF; duplicate into both partition
            # halves (except last iter) so bu runs on 128 partitions
            s_lo = small.tile([NP, DH, N_CAPS], bf16, name=f"slo{h}")
            s_hi = small.tile([NP, DH, N_CAPS], bf16, name=f"shi{h}")
            nc.sync.dma_start(s_lo[0:B], cc_out[r][h][0])
            nc.scalar.dma_start(s_hi[0:B], cc_out[r][h][1])
            if not last:
                nc.scalar.dma_start(s_lo[B:2 * B], cc_out[r][h][0])
                nc.sync.dma_start(s_hi[B:2 * B], cc_out[r][h][1])
            sh = small.tile([NP, DH, N_CAPS], bf16, name=f"sh{h}")
            nc.vector.tensor_add(sh[:], s_lo[:], s_hi[:])
            s_h[h] = sh
            # squash pieces: sq = (inv*s)^2, ss_h = sum_{d in half} sq
            sq = small.tile([NP, DH, N_CAPS], bf16, name=f"sq{h}")
            nc.vector.scalar_tensor_tensor(
                sq[:], sh[:], inv * inv, sh[:],
                mybir.AluOpType.mult, mybir.AluOpType.mult,
            )
            ssh = small.tile([NP, N_CAPS], f32, name=f"ssh{h}")
            with nc.allow_low_precision(
                reason="16-term bf16 square-sum into f32; 0.2% on squash scale"
            ):
                nc.vector.reduce_sum(
                    out=ssh[:], in_=sq.transpose([0, 2, 1]),
                    axis=mybir.AxisListType.X,
                )
            ss_h[h] = ssh
            if not last:
                # bu-mul for this half while the other half's AR flies:
                # sum_d s*u_hat (squash scale deferred to after the fold)
                nc.vector.tensor_mul(
                    tmp[:], u_hat[:, h * DH:(h + 1) * DH],
                    sh.unsqueeze(2).broadcast_to([128, DH, I1, N_CAPS]),
                )
                w = DH
                while w > 2:
                    nc.vector.tensor_add(
                        tmp[:, :w // 2], tmp[:, :w // 2], tmp[:, w // 2:w]
                    )
                    w //= 2
                ph = small.tile([128, I1, N_CAPS], bf16, name=f"part{h}")
                nc.vector.tensor_add(ph[:], tmp[:, 0], tmp[:, 1])
                part[h] = ph
            if h == 0:
                continue
            # both halves returned: finish the squash scale
            # scale = inv * ss/(1+ss)/sqrt(ss+eps), sqrt via exp(0.5*ln)
            ss = small.tile([NP, N_CAPS], f32)
            nc.vector.tensor_add(ss[:], ss_h[0][:], ss_h[1][:])
            t1 = small.tile([NP, N_CAPS], f32)
            nc.scalar.activation(
                out=t1[:], in_=ss[:], func=mybir.ActivationFunctionType.Ln,
                bias=eps_t[0:NP], scale=1.0,
            )
            tq = small.tile([NP, N_CAPS], f32)
            nc.scalar.activation(
                out=tq[:], in_=t1[:], func=mybir.ActivationFunctionType.Exp,
                bias=0.0, scale=0.5,
            )   # sqrt(ss+eps), overlapped with the bu-mul above
            t2 = small.tile([NP, N_CAPS], f32)
            nc.vector.scalar_tensor_tensor(
                t2[:], ss[:], 1.0, tq[:],
                mybir.AluOpType.add, mybir.AluOpType.mult,
            )   # (1+ss)*sqrt(ss+eps)
            nc.vector.reciprocal(out=t2[:], in_=t2[:])
            t1b = small.tile([NP, N_CAPS], bf16)
            nc.vector.scalar_tensor_tensor(
                t1b[:], ss[:], inv, t2[:],
                mybir.AluOpType.mult, mybir.AluOpType.mult,
            )   # squash scale (*inv), bf16

        if last:
            # out[b,j,d] = scale * s via a transposed-AP write
            out_t = small.tile([B, N_CAPS, OUT_DIM], f32)
            for h in (0, 1):
                nc.vector.tensor_mul(
                    out_t[:, :, h * DH:(h + 1) * DH].transpose([0, 2, 1]),
                    s_h[h][:],
                    t1b.unsqueeze(1).broadcast_to([B, DH, N_CAPS]),
                )
            nc.sync.dma_start(out[:], out_t[:])
            break

        # b_log update: b_log += scale * (part0 + part1)
        agr = small.tile([128, I1, N_CAPS], bf16)
        nc.vector.tensor_add(agr[:], part[0][:], part[1][:])
        tb = t1b.unsqueeze(1).broadcast_to([128, I1, N_CAPS])
        if r == 0:
            nc.vector.tensor_mul(b_log[:], agr[:], tb)
        else:
            tmul = small.tile([128, I1, N_CAPS], bf16)
            nc.vector.tensor_mul(tmul[:], agr[:], tb)
            nc.vector.tensor_add(b_log[:], b_log[:], tmul[:])

        # softmax over j: |b| stays < ~20, exp is fp32-safe without the
        # max-subtract. cexp/reciprocal in bf16 keep the DVE 2x mode.
        cexp = small.tile([128, I1, N_CAPS], bf16)
        nc.scalar.activation(
            out=cexp[:], in_=b_log[:],
            func=mybir.ActivationFunctionType.Exp,
        )
        csum = small.tile([128, I1], f32)
        with nc.allow_low_precision(
            reason="16-term bf16 sum into f32; 0.4% noise on c"
        ):
            nc.vector.reduce_sum(
                out=csum[:], in_=cexp[:], axis=mybir.AxisListType.X
            )
        csum_r = small.tile([128, I1], bf16)
        with nc.allow_low_precision(
            reason="bf16 softmax reciprocal; 0.4% noise on c"
        ):
            nc.vector.reciprocal(out=csum_r[:], in_=csum[:])
        c_t = small.tile([128, I1, N_CAPS], bf16)
        nc.vector.tensor_mul(
            c_t[:], cexp[:],
            csum_r.unsqueeze(2).broadcast_to([128, I1, N_CAPS]),
        )
        # s(r+1) partials per d-half; AR triggers as each half folds
        for h in (0, 1):
            nc.vector.tensor_mul(
                tmp[:], u_hat[:, h * DH:(h + 1) * DH],
                c_t.unsqueeze(1).broadcast_to([128, DH, I1, N_CAPS]),
            )
            w = I1
            while w > 2:
                nc.vector.tensor_add(
                    tmp[:, :, :w // 2], tmp[:, :, :w // 2], tmp[:, :, w // 2:w]
                )
                w //= 2
            shh = small.tile([128, DH, N_CAPS], bf16, name=f"shalf{h}")
            nc.vector.tensor_add(shh[:], tmp[:, :, 0, :], tmp[:, :, 1, :])
            nc.sync.dma_start(cc_in[r + 1][h][:], shh[:])
            allreduce(r + 1, h)

    ctx.close()


class _single_act_table:
    """Make every activation resolve to the one table set that covers
    Exp+Ln+Copy (natural_log_exp_and_others), so the kernel loads activation
    tables exactly once instead of thrashing Exp<->Ln sets (~1.3us per
    reload, on the critical path). Positional set ids are preserved, so the
    walrus side (which indexes the same act_info.json) stays consistent.
    Scoped: restores the original resolver on exit."""

    def __enter__(self):
        import concourse.bacc as bacc

        self._bacc = bacc
        self._orig = orig = bacc.get_activation_tables

        def patched(arch):
            tables = dict(orig(arch))
            keep = "natural_log_exp_and_others"
            if keep in tables:
                for k in tables:
                    if k != keep:
                        tables[k] = set()
            return tables

        bacc.get_activation_tables = patched
        return self

    def __exit__(self, *exc):
        self._bacc.get_activation_tables = self._orig
        return False


def _build(num_routing):
    import concourse.bacc as bacc
    import concourse.tile as tile
    from concourse import mybir

    nc = bacc.Bacc(
        "TRN2", target_bir_lowering=False, debug=False, num_devices=N_CORES,
        dynamic_dma_scratch_size=512,
    )
    f32 = mybir.dt.float32
    bf16 = mybir.dt.bfloat16
    xT = nc.dram_tensor("xT", [IN_DIM, I_LOC, B], bf16, kind="ExternalInput")
    wT = nc.dram_tensor(
        "wT", [NPAIR, IN_DIM, 2, GRP, OUT_DIM, N_CAPS], bf16,
        kind="ExternalInput",
    )
    out = nc.dram_tensor(
        "out", [B, N_CAPS, OUT_DIM], f32, kind="ExternalOutput"
    )
    with tile.TileContext(nc) as tc:
        _emit(tc, xT, wT, out, num_routing)
    with _single_act_table():
        nc.compile()
    return nc


def kernel(inputs, W, num_routing):
    import ml_dtypes

    from concourse.bass_utils import run_bass_kernel_spmd

    R = int(num_routing)
    assert R >= 1
    if R not in _cache:
        _cache[R] = _build(R)
    nc = _cache[R]

    bf = ml_dtypes.bfloat16
    inputs = np.ascontiguousarray(np.asarray(inputs, dtype=np.float32))
    W = np.asarray(W, dtype=np.float32)

    in_maps = []
    for c in range(N_CORES):
        lo, hi = c * I_LOC, (c + 1) * I_LOC
        xT_c = np.ascontiguousarray(
            inputs[:, lo:hi, :].transpose(2, 1, 0).astype(bf)
        )
        # [i,j,k,d] -> pair-blocked [p, k, g2, t, d, j]: each 2-group DMA is
        # one contiguous 1MB block with 8KB contiguous per partition line
        wT_c = np.ascontiguousarray(
            W[lo:hi]
            .reshape(NPAIR, 2, GRP, N_CAPS, IN_DIM, OUT_DIM)
            .transpose(0, 4, 1, 2, 5, 3)
            .astype(bf)
        )
        in_maps.append({"xT": xT_c, "wT": wT_c})

    kwargs = {}
    if TRACE:
        kwargs["trace"] = True
        if TRACE_DIR:
            kwargs["tmpdir"] = TRACE_DIR
    res = None
    for attempt in range(3):
        try:
            res = run_bass_kernel_spmd(
                nc, in_maps, core_ids=list(range(N_CORES)), **kwargs
            )
            break
        except Exception:
            if attempt == 2:
                raise
            import time
            time.sleep(5)
    if TRACE:
        kernel.last_exec_time_ns = res.exec_time_ns
        kernel.last_results = res
    return np.asarray(res.results[0]["out"], dtype=np.float32)


# revision 10
# speedup vs baseline: 1.0540x; 1.0044x over previous
"""CapsuleLayer (dynamic routing) Trainium2 kernel, SPMD over 8 NeuronCores.

Sharding: input-capsule axis (IN_CAPS=512 -> 64 per core). W and u_hat are
i-sharded; the bij,bijd->bjd contraction is completed with AllReduces of
bf16 s-partials once per routing iteration.

Per-core layout (i_local = i2*32 + i1, i2 in {0,1}):
  u_hat SBUF [p=(i2*64+b), (d, i1, j)] bf16 -- 128 partitions x 16384
  b/c logits [p, (i1, j)], s partials [p, (d, j)].

v2 structure (over the 245us baseline):
- No warmup AllReduce. Instead, the r=0 s-partial (c uniform -> s0 =
  (1/16) sum_i u_hat, the 1/16 folded into the squash scale) is built
  incrementally DURING phase 1: each 4-i group is folded 4->1 as its
  PSUM cast lands, staged into gfs[128, 8, (d,j)], and tree-folded 8->1
  right after the last cast. Its AllReduce triggers at ~46us, so the AR
  starts the moment the NRT collective-init barrier (48-145us, run
  variance) ends, instead of queueing behind a warmup AR + DVE fold
  (~36us of serialized warmup+AR0 on the baseline critical path).
- Every s AllReduce is split into two d-halves (64 KB bf16 each) on the
  single CC stream. Downstream work is d-decomposable: the agreement
  update b_log += sum_d out*u_hat = scale[b,j] * sum_d s*u_hat (squash
  scale applied AFTER the d-fold, algebraically identical), so the
  bu-mul+fold for half A runs while half B's AR is still in flight.
  Per-iteration AR exposure drops from ~12.5us + full serial DVE to
  mostly-hidden.
- cc buffers are dedicated (bufs = exact tile count) so no DRAM-pool
  aliasing creates false WAR semaphore deps on the AR triggers.
- sqrt(ss+eps) = exp(0.5*ln(ss+eps)) with activation tables pinned to
  the one set containing Ln+Exp+Copy (no per-iteration table reloads).

Measured notes (from traces): DVE tensor_tensor bf16 runs 2x only with
step-1 innermost and full-width ops; broadcast over a non-innermost dim
keeps 2x. AR latency is fixed ~11-13us nearly independent of payload
(128B vs 128KB), so splitting costs stream time but buys overlap. The
NRT barrier (collective stack init) ends 48-145us into the kernel and
gates the first AR; nothing in-kernel controls it. GPSIMD ~1.9ns/elem
flat, ~4.7 sliced/broadcast. remote_dma hangs this runtime (axon
fake_nrt), so the collective stack is unavoidable.

Phase 1 (per i): u_hat_i[b, dj] = xT_i.T @ W_i on the PE (K=128, M=64,
N=512), all in bf16, W streamed in 1 MB pair-blocked chunks.
"""

import numpy as np

N_CORES = 8
B = 64
IN_CAPS = 512
IN_DIM = 128
N_CAPS = 16
OUT_DIM = 32
I_LOC = IN_CAPS // N_CORES          # 64 input capsules per core
I1 = 32                             # i_local = i2*32 + i1
DH = OUT_DIM // 2                   # 16, d-half for split ARs
EPS = 1e-7
GRP = 4                             # i's per PSUM group
NGRP = I_LOC // GRP                 # 16
NPAIR = NGRP // 2                   # 8 (two groups per W DMA)

# Toggled by test.py for profiling runs.
TRACE = False
TRACE_DIR = None

_cache = {}


def _emit(tc, xT, wT, out, num_routing):
    from contextlib import ExitStack

    from concourse import mybir

    nc = tc.nc
    f32 = mybir.dt.float32
    bf16 = mybir.dt.bfloat16
    R = num_routing
    ctx = ExitStack()
    singles = ctx.enter_context(tc.tile_pool(name="singles", bufs=1))
    wpool = ctx.enter_context(tc.tile_pool(name="wpool", bufs=4))
    pspool = ctx.enter_context(tc.tile_pool(name="pspool", bufs=2, space="PSUM"))
    small = ctx.enter_context(tc.tile_pool(name="small", bufs=2))
    gpool = ctx.enter_context(tc.tile_pool(name="gpool", bufs=2))
    # one slot per cc buffer: zero reuse -> no false WAR deps on triggers
    ccpool = ctx.enter_context(tc.tile_pool(name="ccpool", bufs=4 * R, space="DRAM"))

    # r=0 gets ONE full-width AR: its payload is ready long before the NRT
    # collective-init barrier lifts, so a d-split would only serialize two
    # ARs on the stream with nothing to overlap (~10us slower to full s0).
    # r>=1 ARs are d-split so bu-mul(half A) overlaps AR(half B).
    cc_in = [[None, None] for _ in range(R)]
    cc_out = [[None, None] for _ in range(R)]
    cc_in[0][0] = ccpool.tile([2, B, OUT_DIM, N_CAPS], bf16, name="ccin0")
    cc_out[0][0] = ccpool.tile([2, B, OUT_DIM, N_CAPS], bf16, name="ccout0")
    for r in range(1, R):
        for h in (0, 1):
            cc_in[r][h] = ccpool.tile(
                [2, B, DH, N_CAPS], bf16, name=f"ccin{r}{h}"
            )
            cc_out[r][h] = ccpool.tile(
                [2, B, DH, N_CAPS], bf16, name=f"ccout{r}{h}"
            )

    def allreduce(r, h):
        nc.gpsimd.collective_compute(
            "AllReduce",
            mybir.AluOpType.add,
            replica_groups=[list(range(N_CORES))],
            ins=[cc_in[r][h].opt()],
            outs=[cc_out[r][h].opt()],
        )

    # ---- phase 1: u_hat = einsum over k, per local capsule i; the r=0
    # s-partial sum_i u_hat accumulates alongside ----
    xsb = singles.tile([IN_DIM, I_LOC, B], bf16)         # [k, i, b]
    u_hat = singles.tile([128, OUT_DIM, I1, N_CAPS], bf16)  # [(i2,b), d, i1, j]
    gfs = singles.tile([128, NPAIR, OUT_DIM, N_CAPS], bf16)  # group partials
    eps_t = singles.tile([128, 1], f32)
    nc.vector.memset(eps_t[:], EPS)

    XCH = I_LOC // 4
    for p in range(NPAIR):
        # interleave x chunks with the first W pairs so the first matmul's
        # operands land as early as possible. All loads stay on the sync
        # queue (engine queues are in-order; scalar must not stall).
        if p < 4:
            nc.sync.dma_start(
                xsb[:, p * XCH:(p + 1) * XCH, :],
                xT[:, p * XCH:(p + 1) * XCH, :],
            )
        wtile = wpool.tile([IN_DIM, 2, GRP, OUT_DIM, N_CAPS], bf16)
        nc.sync.dma_start(wtile[:], wT[p])
        for g2 in range(2):
            g = 2 * p + g2
            i2 = (g * GRP) // I1
            i1g = (g * GRP) % I1
            gh = (g % NPAIR)  # staging slot within this i2-half
            ps = pspool.tile([128, GRP, OUT_DIM, N_CAPS], f32)
            for t in range(GRP):
                i = g * GRP + t
                nc.tensor.matmul(
                    ps[i2 * B:(i2 + 1) * B, t], xsb[:, i, :], wtile[:, g2, t],
                    start=True, stop=True,
                )
            # copy+cast PSUM f32 -> SBUF bf16, alternating ACT/DVE so the
            # cast chain isn't serialized on one engine at the end of the
            # W stream (GPSIMD cannot read PSUM); dst viewed (i1, d, j)
            dst = u_hat[i2 * B:(i2 + 1) * B, :, i1g:i1g + GRP, :].transpose(
                [0, 2, 1, 3]
            )
            if g % 2 == 0:
                nc.scalar.copy(out=dst, in_=ps[i2 * B:(i2 + 1) * B])
            else:
                nc.vector.tensor_copy(out=dst, in_=ps[i2 * B:(i2 + 1) * B])
            # incremental r0 fold: this group's 4 i's -> gfs slot
            ug = u_hat[i2 * B:(i2 + 1) * B, :, i1g:i1g + GRP, :]
            gtmp = gpool.tile([B, OUT_DIM, 2, N_CAPS], bf16, name="gtmp")
            nc.vector.tensor_add(gtmp[:], ug[:, :, 0:2, :], ug[:, :, 2:4, :])
            nc.vector.tensor_add(
                gfs[i2 * B:(i2 + 1) * B, gh], gtmp[:, :, 0, :], gtmp[:, :, 1, :]
            )
    # tree-fold the 8 group partials per half -> acc0 = sum_i u_hat
    nc.vector.tensor_add(gfs[:, :4], gfs[:, :4], gfs[:, 4:8])
    nc.vector.tensor_add(gfs[:, :2], gfs[:, :2], gfs[:, 2:4])
    acc0 = singles.tile([128, OUT_DIM, N_CAPS], bf16)
    nc.vector.tensor_add(acc0[:], gfs[:, 0], gfs[:, 1])
    nc.sync.dma_start(cc_in[0][0][:], acc0[:])
    allreduce(0, 0)

    # ---- phase 2: routing, d-split pipelined around the ARs ----
    tmp = singles.tile([128, DH, I1, N_CAPS], bf16)
    b_log = singles.tile([128, I1, N_CAPS], f32)

    def squash_sqrt(ss_in, NP):
        # sqrt(ss+eps) via exp(0.5*ln(ss+eps)) on the ACT engine, so Ln/Exp
        # stay in one activation-table set; emitted BEFORE the bu-mul so
        # ACT streams while the DVE is busy on the big multiply
        t1 = small.tile([NP, N_CAPS], f32)
        nc.scalar.activation(
            out=t1[:], in_=ss_in[:], func=mybir.ActivationFunctionType.Ln,
            bias=eps_t[0:NP], scale=1.0,
        )
        tq = small.tile([NP, N_CAPS], f32)
        nc.scalar.activation(
            out=tq[:], in_=t1[:], func=mybir.ActivationFunctionType.Exp,
            bias=0.0, scale=0.5,
        )
        return tq

    def squash_fin(ss_in, tq, NP, inv):
        # scale = inv * ss/(1+ss)/sqrt(ss+eps)
        t2 = small.tile([NP, N_CAPS], f32)
        nc.vector.scalar_tensor_tensor(
            t2[:], ss_in[:], 1.0, tq[:],
            mybir.AluOpType.add, mybir.AluOpType.mult,
        )   # (1+ss)*sqrt(ss+eps)
        nc.vector.reciprocal(out=t2[:], in_=t2[:])
        t1b = small.tile([NP, N_CAPS], bf16)
        nc.vector.scalar_tensor_tensor(
            t1b[:], ss_in[:], inv, t2[:],
            mybir.AluOpType.mult, mybir.AluOpType.mult,
        )   # squash scale (*inv), bf16
        return t1b

    def bu_half(sh, h):
        # sum_{d in half} s*u_hat (squash scale deferred to after the fold:
        # b_log += sum_d out*u = scale[b,j] * sum_d s*u, algebraically equal)
        nc.vector.tensor_mul(
            tmp[:], u_hat[:, h * DH:(h + 1) * DH],
            sh.unsqueeze(2).broadcast_to([128, DH, I1, N_CAPS]),
        )
        w = DH
        while w > 2:
            nc.vector.tensor_add(
                tmp[:, :w // 2], tmp[:, :w // 2], tmp[:, w // 2:w]
            )
            w //= 2
        ph = small.tile([128, I1, N_CAPS], bf16, name=f"part{h}")
        nc.vector.tensor_add(ph[:], tmp[:, 0], tmp[:, 1])
        return ph

    for r in range(R):
        last = r == R - 1
        NP = B if last else 128
        # iteration 0's uniform c = 1/16 is folded into the squash math:
        # with s' = 16*s, ss = sum_d (s'/16)^2 and out = f(ss) * (s'/16)
        inv = 1.0 / N_CAPS if r == 0 else 1.0

        part = [None, None]
        if r == 0:
            # single full-width AR result; duplicate into both partition
            # halves (except last iter) so bu runs on 128 partitions
            s_lo = small.tile([NP, OUT_DIM, N_CAPS], bf16, name="slof")
            s_hi = small.tile([NP, OUT_DIM, N_CAPS], bf16, name="shif")
            nc.sync.dma_start(s_lo[0:B], cc_out[0][0][0])
            nc.scalar.dma_start(s_hi[0:B], cc_out[0][0][1])
            if not last:
                nc.scalar.dma_start(s_lo[B:2 * B], cc_out[0][0][0])
                nc.sync.dma_start(s_hi[B:2 * B], cc_out[0][0][1])
            s_full = small.tile([NP, OUT_DIM, N_CAPS], bf16, name="sfull")
            nc.vector.tensor_add(s_full[:], s_lo[:], s_hi[:])
            s_h = [s_full[:, 0:DH], s_full[:, DH:OUT_DIM]]
            sq = small.tile([NP, OUT_DIM, N_CAPS], bf16, name="sqf")
            nc.vector.scalar_tensor_tensor(
                sq[:], s_full[:], inv * inv, s_full[:],
                mybir.AluOpType.mult, mybir.AluOpType.mult,
            )
            ss = small.tile([NP, N_CAPS], f32)
            with nc.allow_low_precision(
                reason="32-term bf16 square-sum into f32; 0.2% on squash scale"
            ):
                nc.vector.reduce_sum(
                    out=ss[:], in_=sq.transpose([0, 2, 1]),
                    axis=mybir.AxisListType.X,
                )
            tq = squash_sqrt(ss, NP)
            if not last:
                part[0] = bu_half(s_h[0], 0)
                part[1] = bu_half(s_h[1], 1)
            t1b = squash_fin(ss, tq, NP, inv)
        else:
            s_h = [None, None]
            ss_h = [None, None]
            for h in (0, 1):
                # AR(r) half-h result -> SBUF; bu for half 0 runs while
                # half 1's AR is still in flight on the CC stream
                s_lo = small.tile([NP, DH, N_CAPS], bf16, name=f"slo{h}")
                s_hi = small.tile([NP, DH, N_CAPS], bf16, name=f"shi{h}")
                nc.sync.dma_start(s_lo[0:B], cc_out[r][h][0])
                nc.scalar.dma_start(s_hi[0:B], cc_out[r][h][1])
                if not last:
                    nc.scalar.dma_start(s_lo[B:2 * B], cc_out[r][h][0])
                    nc.sync.dma_start(s_hi[B:2 * B], cc_out[r][h][1])
                sh = small.tile([NP, DH, N_CAPS], bf16, name=f"sh{h}")
                nc.vector.tensor_add(sh[:], s_lo[:], s_hi[:])
                s_h[h] = sh
                # squash pieces: sq = (inv*s)^2, ss_h = sum_{d in half} sq
                sq = small.tile([NP, DH, N_CAPS], bf16, name=f"sq{h}")
                nc.vector.scalar_tensor_tensor(
                    sq[:], sh[:], inv * inv, sh[:],
                    mybir.AluOpType.mult, mybir.AluOpType.mult,
                )
                ssh = small.tile([NP, N_CAPS], f32, name=f"ssh{h}")
                with nc.allow_low_precision(
                    reason="16-term bf16 square-sum into f32; 0.2% on scale"
                ):
                    nc.vector.reduce_sum(
                        out=ssh[:], in_=sq.transpose([0, 2, 1]),
                        axis=mybir.AxisListType.X,
                    )
                ss_h[h] = ssh
                if h == 0:
                    if not last:
                        part[0] = bu_half(sh, 0)
                    continue
                ss = small.tile([NP, N_CAPS], f32)
                nc.vector.tensor_add(ss[:], ss_h[0][:], ss_h[1][:])
                tq = squash_sqrt(ss, NP)
                if not last:
                    part[1] = bu_half(sh, 1)
                t1b = squash_fin(ss, tq, NP, inv)

        if last:
            # out[b,j,d] = scale * s via a transposed-AP write
            out_t = small.tile([B, N_CAPS, OUT_DIM], f32)
            for h in (0, 1):
                nc.vector.tensor_mul(
                    out_t[:, :, h * DH:(h + 1) * DH].transpose([0, 2, 1]),
                    s_h[h][:],
                    t1b.unsqueeze(1).broadcast_to([B, DH, N_CAPS]),
                )
            nc.sync.dma_start(out[:], out_t[:])
            break

        # b_log update: b_log += scale * (part0 + part1)
        agr = small.tile([128, I1, N_CAPS], bf16)
        nc.vector.tensor_add(agr[:], part[0][:], part[1][:])
        tb = t1b.unsqueeze(1).broadcast_to([128, I1, N_CAPS])
        if r == 0:
            nc.vector.tensor_mul(b_log[:], agr[:], tb)
        else:
            tmul = small.tile([128, I1, N_CAPS], bf16)
            nc.vector.tensor_mul(tmul[:], agr[:], tb)
            nc.vector.tensor_add(b_log[:], b_log[:], tmul[:])

        # softmax over j: |b| stays < ~20, exp is fp32-safe without the
        # max-subtract. cexp/reciprocal in bf16 keep the DVE 2x mode.
        cexp = small.tile([128, I1, N_CAPS], bf16)
        nc.scalar.activation(
            out=cexp[:], in_=b_log[:],
            func=mybir.ActivationFunctionType.Exp,
        )
        csum = small.tile([128, I1], f32)
        with nc.allow_low_precision(
            reason="16-term bf16 sum into f32; 0.4% noise on c"
        ):
            nc.vector.reduce_sum(
                out=csum[:], in_=cexp[:], axis=mybir.AxisListType.X
            )
        csum_r = small.tile([128, I1], bf16)
        with nc.allow_low_precision(
            reason="bf16 softmax reciprocal; 0.4% noise on c"
        ):
            nc.vector.reciprocal(out=csum_r[:], in_=csum[:])
        c_t = small.tile([128, I1, N_CAPS], bf16)
        nc.vector.tensor_mul(
            c_t[:], cexp[:],
            csum_r.unsqueeze(2).broadcast_to([128, I1, N_CAPS]),
        )
        # s(r+1) partials per d-half; AR triggers as each half folds
        for h in (0, 1):
            nc.vector.tensor_mul(
                tmp[:], u_hat[:, h * DH:(h + 1) * DH],
                c_t.unsqueeze(1).broadcast_to([128, DH, I1, N_CAPS]),
            )
            w = I1
            while w > 2:
                nc.vector.tensor_add(
                    tmp[:, :, :w // 2], tmp[:, :, :w // 2], tmp[:, :, w // 2:w]
                )
                w //= 2
            shh = small.tile([128, DH, N_CAPS], bf16, name=f"shalf{h}")
            nc.vector.tensor_add(shh[:], tmp[:, :, 0, :], tmp[:, :, 1, :])
            nc.sync.dma_start(cc_in[r + 1][h][:], shh[:])
            allreduce(r + 1, h)

    ctx.close()


class _single_act_table:
    """Make every activation resolve to the one table set that covers
    Exp+Ln+Copy (natural_log_exp_and_others), so the kernel loads activation
    tables exactly once instead of thrashing Exp<->Ln sets (~1.3us per
    reload, on the critical path). Positional set ids are preserved, so the
    walrus side (which indexes the same act_info.json) stays consistent.
    Scoped: restores the original resolver on exit."""

    def __enter__(self):
        import concourse.bacc as bacc

        self._bacc = bacc
        self._orig = orig = bacc.get_activation_tables

        def patched(arch):
            tables = dict(orig(arch))
            keep = "natural_log_exp_and_others"
            if keep in tables:
                for k in tables:
                    if k != keep:
                        tables[k] = set()
            return tables

        bacc.get_activation_tables = patched
        return self

    def __exit__(self, *exc):
        self._bacc.get_activation_tables = self._orig
        return False


def _build(num_routing):
    import concourse.bacc as bacc
    import concourse.tile as tile
    from concourse import mybir

    nc = bacc.Bacc(
        "TRN2", target_bir_lowering=False, debug=False, num_devices=N_CORES,
        dynamic_dma_scratch_size=512,
    )
    f32 = mybir.dt.float32
    bf16 = mybir.dt.bfloat16
    xT = nc.dram_tensor("xT", [IN_DIM, I_LOC, B], bf16, kind="ExternalInput")
    wT = nc.dram_tensor(
        "wT", [NPAIR, IN_DIM, 2, GRP, OUT_DIM, N_CAPS], bf16,
        kind="ExternalInput",
    )
    out = nc.dram_tensor(
        "out", [B, N_CAPS, OUT_DIM], f32, kind="ExternalOutput"
    )
    with tile.TileContext(nc) as tc:
        _emit(tc, xT, wT, out, num_routing)
    with _single_act_table():
        nc.compile()
    return nc


def kernel(inputs, W, num_routing):
    import ml_dtypes

    from concourse.bass_utils import run_bass_kernel_spmd

    R = int(num_routing)
    assert R >= 1
    if R not in _cache:
        _cache[R] = _build(R)
    nc = _cache[R]

    bf = ml_dtypes.bfloat16
    inputs = np.ascontiguousarray(np.asarray(inputs, dtype=np.float32))
    W = np.asarray(W, dtype=np.float32)

    in_maps = []
    for c in range(N_CORES):
        lo, hi = c * I_LOC, (c + 1) * I_LOC
        xT_c = np.ascontiguousarray(
            inputs[:, lo:hi, :].transpose(2, 1, 0).astype(bf)
        )
        # [i,j,k,d] -> pair-blocked [p, k, g2, t, d, j]: each 2-group DMA is
        # one contiguous 1MB block with 8KB contiguous per partition line
        wT_c = np.ascontiguousarray(
            W[lo:hi]
            .reshape(NPAIR, 2, GRP, N_CAPS, IN_DIM, OUT_DIM)
            .transpose(0, 4, 1, 2, 5, 3)
            .astype(bf)
        )
        in_maps.append({"xT": xT_c, "wT": wT_c})

    kwargs = {}
    if TRACE:
        kwargs["trace"] = True
        if TRACE_DIR:
            kwargs["tmpdir"] = TRACE_DIR
    res = None
    for attempt in range(3):
        try:
            res = run_bass_kernel_spmd(
                nc, in_maps, core_ids=list(range(N_CORES)), **kwargs
            )
            break
        except Exception:
            if attempt == 2:
                raise
            import time
            time.sleep(5)
    if TRACE:
        kernel.last_exec_time_ns = res.exec_time_ns
        kernel.last_results = res
    return np.asarray(res.results[0]["out"], dtype=np.float32)
